# revision 1
# baseline (speedup 1.0000x reference)
"""Trainium2 Bass kernel for nn_BiLSTM2 (stacked bidirectional LSTM + attention).

Sharding: pure data parallel over the 1024-row batch across 8 NeuronCores.
The host sorts rows by seq_len descending and stripes them across cores
(core c gets sorted rows c::8), so every core sees an identical,
monotonically shrinking active-prefix schedule n_t and masked work is
skipped entirely (the per-step ops only cover the active prefix).

Layouts are "transposed" end to end: feature/hidden dim on the 128 SBUF
partitions, batch on the free axis. The backward direction runs as a
time-descending recurrence over the original sequence with the same
(t < seq_len) prefix condition — exactly equivalent to TF's
reverse_sequence sandwich. LSTM outputs are bf16 slabs that serve directly
as the next layer's matmul moving operand; there are no transposes.

Attention uses a partition-replicated copy of w_att so one matmul yields
the logits row replicated across all partitions; softmax needs no max
subtraction (tanh-bounded logits), and the weighted pooling is an
elementwise multiply plus a stride-T reduction.
"""

import os

import numpy as np
import ml_dtypes

import concourse.bass as bass
import concourse.tile as tile
import concourse.mybir as mybir
from concourse.bass_utils import run_bass_kernel_spmd
from concourse.vector_clock import ScopedClock

BF16 = mybir.dt.float16  # fp16: 1 cyc/row like bf16, 8x the mantissa (all values here are bounded)
F32 = mybir.dt.float32
AF = mybir.ActivationFunctionType
ALU = mybir.AluOpType
AX = mybir.AxisListType

B, T, F, FS, H = 1024, 169, 36, 19, 128
NCORES = 8
BS = B // NCORES  # 128 rows per core
TB = T * BS       # unpacked slab columns

# on-chip gate chunk order: [i, f, o, j] (reference packs i, j, f, o)
_GATE_PERM = np.concatenate([
    np.arange(0, H),          # i
    np.arange(2 * H, 3 * H),  # f
    np.arange(3 * H, 4 * H),  # o
    np.arange(H, 2 * H),      # j
])


def _patch_tile_drain():
    """walrus in this container rejects >1 sem wait on the tail Drain;
    split the waits across extra NoOps (one wait each)."""

    def _drain_and_barrier(self, tick_clock, wait_clock):
        nc = self.nc
        drain_inst = nc.sync.drain()
        wait_clock.add_sem_waits(
            drain_inst.ins, ScopedClock({None: tick_clock.global_clock})
        )
        mi = drain_inst.ins
        si = mi.sync_info
        if si is not None and si.on_wait is not None and len(si.on_wait) > 1:
            waits = list(si.on_wait)
            upd = list(si.on_update) if si.on_update else []
            mi.sync_info = mybir.SyncInfo(on_wait=[waits[0]], on_update=upd)
            for w in waits[1:]:
                nop = nc.sync.nop()
                nop.ins.sync_info = mybir.SyncInfo(on_wait=[w], on_update=[])
        nc.all_engine_barrier()
        popped = nc._tile_sem_poison_stack.pop()
        assert popped is self._sem_poison
        nc.clear_and_free_semaphores(list(self.sems.allocated().values()))
        nc.all_engine_barrier()

    tile.TileContext._drain_and_barrier = _drain_and_barrier


_patch_tile_drain()


def split_multi_waits(nc, max_waits=1):
    """walrus here rejects instructions with more than ~1-2 sem waits; hoist
    extras onto same-engine NoOps placed immediately before the instruction
    (same-engine program order makes this semantically identical)."""
    import bass_rust
    nid = [0]
    for fn in nc.m.functions:
        for blk in fn.blocks:
            out = []
            changed = False
            for inst in blk.instructions:
                si = inst.sync_info
                if si is not None and si.on_wait is not None \
                        and len(si.on_wait) > max_waits:
                    waits = list(si.on_wait)
                    keep = waits[:max_waits]
                    extra = waits[max_waits:]
                    for w in extra:
                        nid[0] += 1
                        nop = bass_rust.InstNoOp(
                            name=f"waitnop-{nid[0]}", ins=[], outs=[],
                            engine=inst.engine)
                        nop.sync_info = mybir.SyncInfo(on_wait=[w],
                                                       on_update=[])
                        out.append(nop)
                    inst.sync_info = mybir.SyncInfo(
                        on_wait=keep,
                        on_update=list(si.on_update) if si.on_update else [])
                    changed = True
                out.append(inst)
            if changed:
                blk.instructions = out
    return nid[0]


def _bf16(x):
    return np.ascontiguousarray(np.asarray(x).astype(np.float16))


def _f32(x):
    return np.ascontiguousarray(np.asarray(x).astype(np.float32))


def build_program(n_sched, off_fw, off_bw, CW, CWB, bias1_chunks, reps=1):
    """Build the SPMD bass program for this call's schedule.

    n_sched[t]: padded active width at step t (multiple of 4, nonincreasing)
    off_fw[t]:  column offset of slab t in packed fw-side buffers (xdT, fw0T);
                slab t is n_sched[t] wide
    off_bw[t]:  offset of slab t in packed bw0T; slab t is
                max(n_sched[t], n_sched[t-1]) wide (the bw recurrence at step
                t-1 reads slab t at width n_sched[t-1])
    bias1_chunks: gate-chunk indices needing a rank-1 bias add in layer 1
    """
    nc = bass.Bass("TRN2", target_bir_lowering=False, debug=False)

    def din(name, shape, dt):
        return nc.dram_tensor(name, shape, dt, kind="ExternalInput").ap()

    xdT = din("xdT", [F + 2, CW], BF16)     # packed [ind_act|ind_mask|f, (t,b)]
    ind2 = din("ind2", [2, CW], BF16)       # indicator rows only (stage 1)
    wx0f = din("wx0f", [F + 2, 512], BF16)
    wh0f = din("wh0f", [H, 512], BF16)
    wx0b = din("wx0b", [F + 2, 512], BF16)
    wh0b = din("wh0b", [H, 512], BF16)
    wx1ft = din("wx1ft", [H, 512], BF16)
    wx1fb = din("wx1fb", [H, 512], BF16)
    wh1f = din("wh1f", [H, 512], BF16)
    wx1bt = din("wx1bt", [H, 512], BF16)
    wx1bb = din("wx1bb", [H, 512], BF16)
    wh1b = din("wh1b", [H, 512], BF16)
    bias1f = din("bias1f", [2, 512], BF16)
    bias1b = din("bias1b", [2, 512], BF16)
    wrepf = din("wrepf", [H, 128], BF16)
    wrepb = din("wrepb", [H, 128], BF16)
    battc = din("battc", [128, 1], F32)
    xsT = din("xsT", [FS, BS], F32)
    ws0 = din("ws0", [FS, 16], F32)
    ws1 = din("ws1", [16, 16], F32)
    bs0 = din("bs0", [16, 1], F32)
    bs1 = din("bs1", [16, 1], F32)
    wc1s = din("wc1s", [16, 64], F32)
    wc1f = din("wc1f", [H, 64], F32)
    wc1b = din("wc1b", [H, 64], F32)
    bc1 = din("bc1", [64, 1], F32)
    wc2 = din("wc2", [64, 32], F32)
    bc2 = din("bc2", [32, 1], F32)

    outT = nc.dram_tensor("outT", [32, BS], F32, kind="ExternalOutput").ap()

    steps = [t for t in range(T) if n_sched[t] > 0]
    ns = len(steps)

    from contextlib import ExitStack

    with tile.TileContext(nc) as tc, ExitStack() as ctx:
        for _rep in range(reps):
            with ExitStack() as rctx:
                persist = rctx.enter_context(tc.tile_pool(name="persist", bufs=1))

                fw1T = persist.tile([H, TB], BF16, tag="fw1T")
                bw1T = persist.tile([H, TB], BF16, tag="bw1T")
                s_wrepf = persist.tile([H, 128], BF16, tag="wrepf")
                s_wrepb = persist.tile([H, 128], BF16, tag="wrepb")
                s_batt = persist.tile([128, 1], F32, tag="batt")
                nc.sync.dma_start(out=s_wrepf[:], in_=wrepf[:])
                nc.sync.dma_start(out=s_wrepb[:], in_=wrepb[:])
                nc.sync.dma_start(out=s_batt[:], in_=battc[:])

                s_cls = {}
                for nm, ap_, shp in (
                    ("xsT", xsT, [FS, BS]), ("ws0", ws0, [FS, 16]),
                    ("ws1", ws1, [16, 16]), ("bs0", bs0, [16, 1]),
                    ("bs1", bs1, [16, 1]), ("wc1s", wc1s, [16, 64]),
                    ("wc1f", wc1f, [H, 64]), ("wc1b", wc1b, [H, 64]),
                    ("bc1", bc1, [64, 1]), ("wc2", wc2, [64, 32]),
                    ("bc2", bc2, [32, 1]),
                ):
                    s_cls[nm] = persist.tile(shp, F32, tag="cls_" + nm, name="cls_" + nm)
                    nc.sync.dma_start(out=s_cls[nm][:], in_=ap_[:])

                sT = persist.tile([16, BS], F32, tag="sT")
                att_f = persist.tile([H, BS], F32, tag="att_f")
                att_b = persist.tile([H, BS], F32, tag="att_b")

                # ---- static branch (tiny) ----
                with tc.tile_pool(name="ps_static", bufs=1, space="PSUM") as pss:
                    ps1 = pss.tile([16, BS], F32, tag="pst1")
                    nc.tensor.matmul(ps1[:], s_cls["ws0"][:], s_cls["xsT"][:],
                                     start=True, stop=True)
                    s0 = persist.tile([16, BS], F32, tag="s0tmp")
                    nc.scalar.activation(s0[:], ps1[:], AF.Relu, bias=s_cls["bs0"][:])
                    ps2 = pss.tile([16, BS], F32, tag="pst2")
                    nc.tensor.matmul(ps2[:], s_cls["ws1"][:], s0[:], start=True, stop=True)
                    nc.scalar.activation(sT[:], ps2[:], AF.Relu, bias=s_cls["bs1"][:])

                nc.gpsimd.memset(fw1T[:], 0.0)
                nc.gpsimd.memset(bw1T[:], 0.0)

                # ================= stages 0 and 1 =================
                # attention accumulators (partials summed in during stage 1)
                den_acc = persist.tile([128, BS], F32, tag="den_acc")
                attf_acc = persist.tile([128, BS], F32, tag="attf_acc")
                attb_acc = persist.tile([128, BS], F32, tag="attb_acc")
                nc.gpsimd.memset(den_acc[:], 0.0)
                nc.gpsimd.memset(attf_acc[:], 0.0)
                nc.gpsimd.memset(attb_acc[:], 0.0)

                with ExitStack() as sctx:
                    p01 = sctx.enter_context(tc.tile_pool(name="stageA", bufs=1))
                    gates = sctx.enter_context(tc.tile_pool(name="gates", bufs=3))
                    psum = sctx.enter_context(
                        tc.tile_pool(name="psum01", bufs=4, space="PSUM"))

                    fw0T = p01.tile([H, CW], BF16, tag="fw0T")
                    bw0T = p01.tile([H, CWB], BF16, tag="bw0T")
                    s_ind2 = p01.tile([2, CW], BF16, tag="ind2")
                    hzero = p01.tile([H, BS], BF16, tag="hzero")
                    nc.gpsimd.memset(hzero[:], 0.0)
                    nc.gpsimd.memset(bw0T[:], 0.0)
                    nc.sync.dma_start(out=s_ind2[:], in_=ind2[:])

                    s_w = {}
                    for nm, ap_, shp in (
                        ("wx0f", wx0f, [F + 2, 512]), ("wh0f", wh0f, [H, 512]),
                        ("wx0b", wx0b, [F + 2, 512]), ("wh0b", wh0b, [H, 512]),
                        ("wx1ft", wx1ft, [H, 512]), ("wx1fb", wx1fb, [H, 512]),
                        ("wh1f", wh1f, [H, 512]), ("wx1bt", wx1bt, [H, 512]),
                        ("wx1bb", wx1bb, [H, 512]), ("wh1b", wh1b, [H, 512]),
                        ("bias1f", bias1f, [2, 512]), ("bias1b", bias1b, [2, 512]),
                    ):
                        s_w[nm] = p01.tile(shp, BF16, tag="w_" + nm, name="w_" + nm)
                        nc.sync.dma_start(out=s_w[nm][:], in_=ap_[:])

                    # c-state: one [H, 2*BS] tile per stage per phase; fw in cols
                    # [0:BS], bw in cols [BS:2BS], so one tanh covers both directions.
                    cpair = {}
                    for st in ("0", "1"):
                        cpair[st] = [p01.tile([H, 2 * BS], F32, tag=f"cp_{st}_{k}",
                                              name=f"cp_{st}_{k}") for k in range(2)]
                        for k in range(2):
                            nc.gpsimd.memset(cpair[st][k][:], 0.0)
                    cstate = {
                        "f0": [cpair["0"][k][:, 0:BS] for k in range(2)],
                        "b0": [cpair["0"][k][:, BS:2 * BS] for k in range(2)],
                        "f1": [cpair["1"][k][:, 0:BS] for k in range(2)],
                        "b1": [cpair["1"][k][:, BS:2 * BS] for k in range(2)],
                    }

                    def lstm_step(name, idx, n, xmm, wh, out_ap, h_prev_ap, bias_mms):
                        """Matmuls + sigmoid + c update for one direction; returns the
                        sigmoid tile for the shared pair tail."""
                        cs = cstate[name]
                        c_prev, c_cur = cs[idx % 2], cs[(idx + 1) % 2]
                        ps = psum.tile([128, 512], F32, tag="ps")
                        # x contributions and bias rank-1s first (no dependency on the
                        # previous step's h), recurrent Wh matmuls last: PE drains the
                        # x-part while waiting for h, leaving only 4 matmuls on the
                        # recurrence critical path.
                        mms = []
                        for ch in range(4):
                            sl = slice(ch * 128, ch * 128 + n)
                            for w_, rhs in xmm:
                                mms.append((sl, w_[:, ch * 128:(ch + 1) * 128], rhs))
                        for bt, ch, ones_rhs in bias_mms:
                            mms.append((slice(ch * 128, ch * 128 + n),
                                        bt[:, ch * 128:(ch + 1) * 128], ones_rhs))
                        for ch in range(4):
                            sl = slice(ch * 128, ch * 128 + n)
                            mms.append((sl, wh[:, ch * 128:(ch + 1) * 128], h_prev_ap))
                        for i, (sl, lhsT, rhs) in enumerate(mms):
                            nc.tensor.matmul(ps[:, sl], lhsT, rhs,
                                             start=(i == 0), stop=(i == len(mms) - 1))
                        sig = gates.tile([128, 512], F32, tag="sig")
                        ap_in = ps[:].rearrange("p (c n) -> p c n", c=4)[:, :, 0:n]
                        ap_out = sig[:].rearrange("p (c n) -> p c n", c=4)[:, :, 0:n]
                        nc.scalar.activation(ap_out, ap_in, AF.Sigmoid)
                        # c' = c*sig(f) + sig(i)*tanh(j), with tanh(j) = 2*sig(2j)-1
                        # (2x folded into weights):
                        #   t1 = sig(f) * c
                        #   u  = (sig(2j) - 0.5) * sig(i)      [= sig(i)*tanh(j)/2]
                        #   c' = 2*u + t1
                        t1 = gates.tile([128, BS], F32, tag="t1")
                        nc.gpsimd.tensor_tensor(t1[:, 0:n], sig[:, 128:128 + n],
                                                c_prev[:, 0:n], ALU.mult)
                        t2 = gates.tile([128, BS], F32, tag="t2")
                        nc.vector.scalar_tensor_tensor(t2[:, 0:n], sig[:, 384:384 + n],
                                                       0.5, sig[:, 0:n],
                                                       ALU.subtract, ALU.mult)
                        nc.vector.scalar_tensor_tensor(c_cur[:, 0:n], t2[:, 0:n],
                                                       2.0, t1[:, 0:n],
                                                       ALU.mult, ALU.add)
                        tcn = gates.tile([128, BS], F32, tag="tcn")
                        nc.scalar.activation(tcn[:, 0:n], c_cur[:, 0:n], AF.Tanh)
                        nc.vector.tensor_tensor(out_ap, tcn[:, 0:n],
                                                sig[:, 256:256 + n], ALU.mult)

                    # ---------- stage 0 (fw0 and bw0 interleaved) ----------
                    with tc.tile_pool(name="poolxd", bufs=1) as poolxd:
                        s_xdT = poolxd.tile([F + 2, CW], BF16, tag="xdT")
                        # xdT loaded in alternating head/tail chunks: fw needs the
                        # head first, bw needs the tail first.
                        NCH = 16
                        bounds = [round(i * CW / NCH) for i in range(NCH + 1)]
                        lo, hi = 0, NCH - 1
                        order = []
                        while lo <= hi:
                            order.append(lo)
                            if hi != lo:
                                order.append(hi)
                            lo += 1
                            hi -= 1
                        for ci in order:
                            a, b_ = bounds[ci], bounds[ci + 1]
                            if b_ > a:
                                nc.sync.dma_start(out=s_xdT[:, a:b_], in_=xdT[:, a:b_])

                        for k in range(ns):
                            t = steps[k]
                            n = n_sched[t]
                            hpf = (hzero[:, 0:n] if k == 0 else
                                   fw0T[:, off_fw[steps[k - 1]]:off_fw[steps[k - 1]] + n])
                            lstm_step(
                                "f0", k, n,
                                [(s_w["wx0f"], s_xdT[:, off_fw[t]:off_fw[t] + n])],
                                s_w["wh0f"], fw0T[:, off_fw[t]:off_fw[t] + n], hpf, [])
                            tb = steps[ns - 1 - k]
                            nb = n_sched[tb]
                            hpb = (hzero[:, 0:nb] if k == 0 else
                                   bw0T[:, off_bw[steps[ns - k]]:off_bw[steps[ns - k]] + nb])
                            lstm_step(
                                "b0", k, nb,
                                [(s_w["wx0b"], s_xdT[:, off_fw[tb]:off_fw[tb] + nb])],
                                s_w["wh0b"], bw0T[:, off_bw[tb]:off_bw[tb] + nb], hpb, [])

                    # ---------- stage 1 (fw1/bw1) + attention chunks ----------
                    with ExitStack() as actx:
                        pat = actx.enter_context(tc.tile_pool(name="attnp", bufs=2))
                        psa = actx.enter_context(
                            tc.tile_pool(name="psum_attn", bufs=2, space="PSUM"))

                        for k in range(ns):
                            t = steps[k]
                            n = n_sched[t]
                            ones_f = s_ind2[0:2, off_fw[t]:off_fw[t] + n]
                            hpf = (hzero[:, 0:n] if k == 0 else
                                   fw1T[:, steps[k - 1] * BS:steps[k - 1] * BS + n])
                            lstm_step(
                                "f1", k, n,
                                [(s_w["wx1ft"], fw0T[:, off_fw[t]:off_fw[t] + n]),
                                 (s_w["wx1fb"], bw0T[:, off_bw[t]:off_bw[t] + n])],
                                s_w["wh1f"], fw1T[:, t * BS:t * BS + n], hpf,
                                [(s_w["bias1f"], ch, ones_f) for ch in bias1_chunks])
                            tb = steps[ns - 1 - k]
                            nb = n_sched[tb]
                            ones_b = s_ind2[0:2, off_fw[tb]:off_fw[tb] + nb]
                            hpb = (hzero[:, 0:nb] if k == 0 else
                                   bw1T[:, steps[ns - k] * BS:steps[ns - k] * BS + nb])
                            lstm_step(
                                "b1", k, nb,
                                [(s_w["wx1bt"], fw0T[:, off_fw[tb]:off_fw[tb] + nb]),
                                 (s_w["wx1bb"], bw0T[:, off_bw[tb]:off_bw[tb] + nb])],
                                s_w["wh1b"], bw1T[:, tb * BS:tb * BS + nb], hpb,
                                [(s_w["bias1b"], ch, ones_b) for ch in bias1_chunks])

                        # attention super-chunks (16 slabs each), emitted in slab
                        # readiness order so they fill stage-1 engine bubbles.
                        G = 16
                        scs = []
                        for t0 in range(0, T, G):
                            g = min(G, T - t0)
                            ready = max(t0 + g - 1, ns - 1 - t0)
                            scs.append((ready, t0, g))
                        scs.sort()
                        for ready, t0, g in scs:
                            c0 = t0 * BS
                            cw_full = g * BS
                            es_sc = pat.tile([128, G * BS], BF16, tag="es_sc")
                            th_sc = pat.tile([128, G * BS], BF16, tag="th_sc", bufs=1)
                            # logits in 512-col blocks (one psum bank each); exp is done
                            # once per super-chunk: Exp lives in a different ACT
                            # table-set than Sigmoid/Tanh and each switch costs ~1.3us.
                            b0 = 0
                            while b0 < cw_full:
                                bw_ = min(512, cw_full - b0)
                                psl = psa.tile([128, 512], F32, tag="psl")
                                nc.tensor.matmul(psl[:, 0:bw_], s_wrepf[:],
                                                 fw1T[:, c0 + b0:c0 + b0 + bw_],
                                                 start=True, stop=False)
                                nc.tensor.matmul(psl[:, 0:bw_], s_wrepb[:],
                                                 bw1T[:, c0 + b0:c0 + b0 + bw_],
                                                 start=False, stop=True)
                                nc.scalar.activation(th_sc[:, b0:b0 + bw_], psl[:, 0:bw_],
                                                     AF.Tanh, bias=s_batt[:])
                                b0 += bw_
                            nc.scalar.activation(es_sc[:, 0:cw_full], th_sc[:, 0:cw_full],
                                                 AF.Exp)
                            es3 = es_sc[:, 0:cw_full].rearrange("p (t b) -> p b t", t=g)
                            part = pat.tile([128, BS], F32, tag="part")
                            nc.vector.tensor_reduce(part[:], es3, AX.X, ALU.add)
                            nc.vector.tensor_tensor(den_acc[:], den_acc[:], part[:],
                                                    ALU.add)
                            for accT, srcT in ((attf_acc, fw1T), (attb_acc, bw1T)):
                                ws = pat.tile([128, G * BS], BF16, tag="ws")
                                nc.vector.tensor_tensor(ws[:, 0:cw_full],
                                                        srcT[:, c0:c0 + cw_full],
                                                        es_sc[:, 0:cw_full], ALU.mult)
                                ws3 = ws[:, 0:cw_full].rearrange("p (t b) -> p b t", t=g)
                                part2 = pat.tile([128, BS], F32, tag="part2")
                                nc.vector.tensor_reduce(part2[:], ws3, AX.X, ALU.add)
                                nc.vector.tensor_tensor(accT[:], accT[:], part2[:],
                                                        ALU.add)

                # ================= normalize + classifier =================
                with ExitStack() as cctx:
                    pcl = cctx.enter_context(tc.tile_pool(name="cls", bufs=1))
                    psc = cctx.enter_context(
                        tc.tile_pool(name="psum_cls", bufs=1, space="PSUM"))
                    rd = pcl.tile([128, BS], F32, tag="rd")
                    nc.vector.reciprocal(rd[:], den_acc[:])
                    nc.vector.tensor_tensor(att_f[:], attf_acc[:], rd[:], ALU.mult)
                    nc.vector.tensor_tensor(att_b[:], attb_acc[:], rd[:], ALU.mult)

                    ph = psc.tile([64, BS], F32, tag="ph")
                    nc.tensor.matmul(ph[:], s_cls["wc1s"][:], sT[:], start=True,
                                     stop=False)
                    nc.tensor.matmul(ph[:], s_cls["wc1f"][:], att_f[:], start=False,
                                     stop=False)
                    nc.tensor.matmul(ph[:], s_cls["wc1b"][:], att_b[:], start=False,
                                     stop=True)
                    h1 = pcl.tile([64, BS], F32, tag="h1")
                    nc.scalar.activation(h1[:], ph[:], AF.Relu, bias=s_cls["bc1"][:])
                    po = psc.tile([32, BS], F32, tag="po")
                    nc.tensor.matmul(po[:], s_cls["wc2"][:], h1[:], start=True,
                                     stop=True)
                    oT = pcl.tile([32, BS], F32, tag="oT")
                    nc.scalar.activation(oT[:], po[:], AF.Relu, bias=s_cls["bc2"][:])
                    nc.sync.dma_start(out=outT[:], in_=oT[:])

    return nc


KILL = -30.0  # sigma(-30) ~ 1e-13: forces h,c to exact 0 (fp16) for ended rows


def prepare(inputs):
    """Compute the per-call schedule and the 8 per-core input maps."""
    L = np.asarray(inputs["seq_len"]).astype(np.int64)
    order = np.argsort(-L, kind="stable")
    perms = [order[c::NCORES] for c in range(NCORES)]

    n_true = np.zeros((NCORES, T), dtype=np.int64)
    for c in range(NCORES):
        Lc = L[perms[c]]
        n_true[c] = (Lc[None, :].T > np.arange(T)[None, :]).sum(axis=0)
    n_sched = n_true.max(axis=0)
    n_sched = np.minimum(((n_sched + 3) // 4) * 4, BS).astype(np.int64)

    off_fw = np.zeros(T, dtype=np.int64)
    acc = 0
    for t in range(T):
        off_fw[t] = acc
        acc += int(n_sched[t])
    CW = int(max(acc, 4))
    off_bw = np.zeros(T, dtype=np.int64)
    acc = 0
    for t in range(T):
        off_bw[t] = acc
        acc += int(max(n_sched[t], n_sched[t - 1] if t > 0 else 0))
    CWB = int(max(acc, 4))

    # layer-1 rank-1 rows: row0 = bias (+forget 1), row1 = kill row
    kill_row = np.zeros(512)
    kill_row[0:384] = KILL  # i, f, o chunks; j untouched
    b1f_row = np.asarray(inputs["bb_f1"], dtype=np.float64)[_GATE_PERM].copy()
    b1b_row = np.asarray(inputs["bb_b1"], dtype=np.float64)[_GATE_PERM].copy()
    b1f_row[H:2 * H] += 1.0
    b1b_row[H:2 * H] += 1.0
    bias_chunks = [ch for ch in range(4)
                   if np.any(b1f_row[ch * H:(ch + 1) * H] != 0)
                   or np.any(b1b_row[ch * H:(ch + 1) * H] != 0)]
    l1_chunks = sorted(set(bias_chunks) | {0, 1, 2})

    def jscale(w):
        w = np.array(w, dtype=np.float64)
        w[..., 3 * H:4 * H] *= 2.0  # fold tanh(j)=2*sigma(2j)-1
        return w

    def stage_wx0(wx, bb):
        w = np.asarray(wx, dtype=np.float64)[:, _GATE_PERM]
        bias = np.asarray(bb, dtype=np.float64)[_GATE_PERM].copy()
        bias[H:2 * H] += 1.0
        return _bf16(jscale(np.concatenate([bias[None, :], kill_row[None, :], w],
                                           axis=0)))

    base_map = dict(
        wx0f=stage_wx0(inputs["Wx_f0"], inputs["bb_f0"]),
        wx0b=stage_wx0(inputs["Wx_b0"], inputs["bb_b0"]),
        wh0f=_bf16(jscale(np.asarray(inputs["Wh_f0"])[:, _GATE_PERM])),
        wh0b=_bf16(jscale(np.asarray(inputs["Wh_b0"])[:, _GATE_PERM])),
        wx1ft=_bf16(jscale(np.asarray(inputs["Wx_f1"])[0:H, _GATE_PERM])),
        wx1fb=_bf16(jscale(np.asarray(inputs["Wx_f1"])[H:2 * H, _GATE_PERM])),
        wh1f=_bf16(jscale(np.asarray(inputs["Wh_f1"])[:, _GATE_PERM])),
        wx1bt=_bf16(jscale(np.asarray(inputs["Wx_b1"])[0:H, _GATE_PERM])),
        wx1bb=_bf16(jscale(np.asarray(inputs["Wx_b1"])[H:2 * H, _GATE_PERM])),
        wh1b=_bf16(jscale(np.asarray(inputs["Wh_b1"])[:, _GATE_PERM])),
        bias1f=_bf16(jscale(np.stack([b1f_row, kill_row]))),
        bias1b=_bf16(jscale(np.stack([b1b_row, kill_row]))),
        wrepf=_bf16(np.repeat(np.asarray(inputs["w_att"])[0:H, 0:1], 128, axis=1)),
        wrepb=_bf16(np.repeat(np.asarray(inputs["w_att"])[H:2 * H, 0:1], 128, axis=1)),
        battc=_f32(np.full((128, 1),
                           float(np.asarray(inputs["b_att"]).reshape(-1)[0]))),
        ws0=_f32(inputs["w_s0"]), ws1=_f32(inputs["w_s1"]),
        bs0=_f32(np.asarray(inputs["b_s0"]).reshape(-1, 1)),
        bs1=_f32(np.asarray(inputs["b_s1"]).reshape(-1, 1)),
        wc1s=_f32(np.asarray(inputs["w_c1"])[0:16]),
        wc1f=_f32(np.asarray(inputs["w_c1"])[16:16 + H]),
        wc1b=_f32(np.asarray(inputs["w_c1"])[16 + H:16 + 2 * H]),
        bc1=_f32(np.asarray(inputs["b_c1"]).reshape(-1, 1)),
        wc2=_f32(inputs["w_c2"]),
        bc2=_f32(np.asarray(inputs["b_c2"]).reshape(-1, 1)),
    )

    in_maps = []
    for c in range(NCORES):
        p = perms[c]
        Lc = L[p]
        xc = np.asarray(inputs["x_dynamic"])[p].astype(np.float32)
        tmask = (np.arange(T)[None, :] < Lc[:, None])
        xc = np.where(tmask[:, :, None], xc, 0.0)
        xcT = xc.transpose(2, 1, 0)
        xdT_h = np.zeros((F + 2, CW), dtype=np.float32)
        for t in range(T):
            n = int(n_sched[t])
            if n == 0:
                continue
            o = int(off_fw[t])
            nt = min(int(n_true[c, t]), n)
            xdT_h[2:F + 2, o:o + n] = xcT[:, t, 0:n]
            xdT_h[0, o:o + nt] = 1.0
            xdT_h[1, o + nt:o + n] = 1.0
        m = dict(base_map)
        m["xdT"] = _bf16(xdT_h)
        m["ind2"] = _bf16(xdT_h[0:2])
        m["xsT"] = _f32(np.asarray(inputs["x_static"])[p].T)
        in_maps.append(m)

    sched = dict(n_sched=n_sched, off_fw=off_fw, off_bw=off_bw, CW=CW,
                 CWB=CWB, l1_chunks=l1_chunks, perms=perms, n_true=n_true)
    return sched, in_maps


def kernel(x_static, x_dynamic, seq_len, w_s0, b_s0, w_s1, b_s1,
           Wx_f0, Wh_f0, bb_f0, Wx_b0, Wh_b0, bb_b0,
           Wx_f1, Wh_f1, bb_f1, Wx_b1, Wh_b1, bb_b1,
           w_att, b_att, w_c1, b_c1, w_c2, b_c2):
    inputs = dict(
        x_static=x_static, x_dynamic=x_dynamic, seq_len=seq_len,
        w_s0=w_s0, b_s0=b_s0, w_s1=w_s1, b_s1=b_s1,
        Wx_f0=Wx_f0, Wh_f0=Wh_f0, bb_f0=bb_f0,
        Wx_b0=Wx_b0, Wh_b0=Wh_b0, bb_b0=bb_b0,
        Wx_f1=Wx_f1, Wh_f1=Wh_f1, bb_f1=bb_f1,
        Wx_b1=Wx_b1, Wh_b1=Wh_b1, bb_b1=bb_b1,
        w_att=w_att, b_att=b_att, w_c1=w_c1, b_c1=b_c1,
        w_c2=w_c2, b_c2=b_c2,
    )
    sched, in_maps = prepare(inputs)
    nc = build_program(sched["n_sched"], sched["off_fw"], sched["off_bw"],
                       sched["CW"], sched["CWB"], sched["l1_chunks"])
    split_multi_waits(nc, max_waits=1)

    trace = os.environ.get("TRN_KERNEL_TRACE", "0") == "1"
    try:
        res = run_bass_kernel_spmd(nc, in_maps, list(range(NCORES)), trace=trace)
    except ModuleNotFoundError:
        # NTFF profiling hook unavailable in this container
        res = run_bass_kernel_spmd(nc, in_maps, list(range(NCORES)))
    if trace:
        kernel.last_results = res
        print(f"[kernel] exec_time_ns={res.exec_time_ns} "
              f"mean={res.mean_exec_time_ns}")

    out = np.zeros((B, 32), dtype=np.float32)
    for c in range(NCORES):
        out[sched["perms"][c]] = res.results[c]["outT"].T
    return out



# revision 15
# speedup vs baseline: 1.0764x; 1.0764x over previous
"""Trainium2 Bass kernel for nn_BiLSTM2 (stacked bidirectional LSTM + attention).

Sharding: pure data parallel over the 1024-row batch across 8 NeuronCores.
The host sorts rows by seq_len descending and stripes them across cores
(core c gets sorted rows c::8), so every core sees an identical,
monotonically shrinking active-prefix schedule n_t and masked work is
skipped entirely (the per-step ops only cover the active prefix).

Layouts are "transposed" end to end: feature/hidden dim on the 128 SBUF
partitions, batch on the free axis. The backward direction runs as a
time-descending recurrence over the original sequence with the same
(t < seq_len) prefix condition — exactly equivalent to TF's
reverse_sequence sandwich. LSTM outputs are bf16 slabs that serve directly
as the next layer's matmul moving operand; there are no transposes.

Attention uses a partition-replicated copy of w_att so one matmul yields
the logits row replicated across all partitions; softmax needs no max
subtraction (tanh-bounded logits), and the weighted pooling is an
elementwise multiply plus a stride-T reduction.
"""

import os

import numpy as np
import ml_dtypes

import concourse.bass as bass
import concourse.tile as tile
import concourse.mybir as mybir
from concourse.bass_utils import run_bass_kernel_spmd
from concourse.vector_clock import ScopedClock

BF16 = mybir.dt.float16  # fp16: 1 cyc/row like bf16, 8x the mantissa (all values here are bounded)
F32 = mybir.dt.float32
AF = mybir.ActivationFunctionType
ALU = mybir.AluOpType
AX = mybir.AxisListType

B, T, F, FS, H = 1024, 169, 36, 19, 128
NCORES = 8
BS = B // NCORES  # 128 rows per core
TB = T * BS       # unpacked slab columns

# on-chip gate chunk order: [i, f, o, j] (reference packs i, j, f, o)
_GATE_PERM = np.concatenate([
    np.arange(0, H),          # i
    np.arange(2 * H, 3 * H),  # f
    np.arange(3 * H, 4 * H),  # o
    np.arange(H, 2 * H),      # j
])


def _patch_tile_drain():
    """walrus in this container rejects >1 sem wait on the tail Drain;
    split the waits across extra NoOps (one wait each)."""

    def _drain_and_barrier(self, tick_clock, wait_clock):
        nc = self.nc
        drain_inst = nc.sync.drain()
        wait_clock.add_sem_waits(
            drain_inst.ins, ScopedClock({None: tick_clock.global_clock})
        )
        mi = drain_inst.ins
        si = mi.sync_info
        if si is not None and si.on_wait is not None and len(si.on_wait) > 1:
            waits = list(si.on_wait)
            upd = list(si.on_update) if si.on_update else []
            mi.sync_info = mybir.SyncInfo(on_wait=[waits[0]], on_update=upd)
            for w in waits[1:]:
                nop = nc.sync.nop()
                nop.ins.sync_info = mybir.SyncInfo(on_wait=[w], on_update=[])
        nc.all_engine_barrier()
        popped = nc._tile_sem_poison_stack.pop()
        assert popped is self._sem_poison
        nc.clear_and_free_semaphores(list(self.sems.allocated().values()))
        nc.all_engine_barrier()

    tile.TileContext._drain_and_barrier = _drain_and_barrier


_patch_tile_drain()


def split_multi_waits(nc, max_waits=1):
    """walrus here rejects instructions with more than ~1-2 sem waits; hoist
    extras onto same-engine NoOps placed immediately before the instruction
    (same-engine program order makes this semantically identical)."""
    import bass_rust
    nid = [0]
    for fn in nc.m.functions:
        for blk in fn.blocks:
            out = []
            changed = False
            for inst in blk.instructions:
                si = inst.sync_info
                if si is not None and si.on_wait is not None \
                        and len(si.on_wait) > max_waits:
                    waits = list(si.on_wait)
                    keep = waits[:max_waits]
                    extra = waits[max_waits:]
                    for w in extra:
                        nid[0] += 1
                        nop = bass_rust.InstNoOp(
                            name=f"waitnop-{nid[0]}", ins=[], outs=[],
                            engine=inst.engine)
                        nop.sync_info = mybir.SyncInfo(on_wait=[w],
                                                       on_update=[])
                        out.append(nop)
                    inst.sync_info = mybir.SyncInfo(
                        on_wait=keep,
                        on_update=list(si.on_update) if si.on_update else [])
                    changed = True
                out.append(inst)
            if changed:
                blk.instructions = out
    return nid[0]


def _bf16(x):
    return np.ascontiguousarray(np.asarray(x).astype(np.float16))


def _f32(x):
    return np.ascontiguousarray(np.asarray(x).astype(np.float32))


def build_program(n_sched, off_fw, off_bw, CW, CWB, bias1_chunks, reps=1):
    """Build the SPMD bass program for this call's schedule.

    n_sched[t]: padded active width at step t (multiple of 4, nonincreasing)
    off_fw[t]:  column offset of slab t in packed fw-side buffers (xdT, fw0T);
                slab t is n_sched[t] wide
    off_bw[t]:  offset of slab t in packed bw0T; slab t is
                max(n_sched[t], n_sched[t-1]) wide (the bw recurrence at step
                t-1 reads slab t at width n_sched[t-1])
    bias1_chunks: gate-chunk indices needing a rank-1 bias add in layer 1
    """
    nc = bass.Bass("TRN2", target_bir_lowering=False, debug=False)

    def din(name, shape, dt):
        return nc.dram_tensor(name, shape, dt, kind="ExternalInput").ap()

    xdT = din("xdT", [F + 2, CW], BF16)     # packed [ind_act|ind_mask|f, (t,b)]
    ind2 = din("ind2", [2, CW], BF16)       # indicator rows only (stage 1)
    zerosT = din("zerosT", [128, TB], BF16)  # DMA'd zeros (slab init)
    ident = din("ident", [128, 128], BF16)   # identity (PE t-reduction)
    wx0f = din("wx0f", [F + 2, 512], BF16)
    wh0f = din("wh0f", [H, 512], BF16)
    wx0b = din("wx0b", [F + 2, 512], BF16)
    wh0b = din("wh0b", [H, 512], BF16)
    wx1ft = din("wx1ft", [H, 512], BF16)
    wx1fb = din("wx1fb", [H, 512], BF16)
    wh1f = din("wh1f", [H, 512], BF16)
    wx1bt = din("wx1bt", [H, 512], BF16)
    wx1bb = din("wx1bb", [H, 512], BF16)
    wh1b = din("wh1b", [H, 512], BF16)
    bias1f = din("bias1f", [2, 512], BF16)
    bias1b = din("bias1b", [2, 512], BF16)
    wrepf = din("wrepf", [H, 128], BF16)
    wrepb = din("wrepb", [H, 128], BF16)
    battc = din("battc", [128, 1], F32)
    xsT = din("xsT", [FS, BS], F32)
    ws0 = din("ws0", [FS, 16], F32)
    ws1 = din("ws1", [16, 16], F32)
    bs0 = din("bs0", [16, 1], F32)
    bs1 = din("bs1", [16, 1], F32)
    wc1s = din("wc1s", [16, 64], F32)
    wc1f = din("wc1f", [H, 64], F32)
    wc1b = din("wc1b", [H, 64], F32)
    bc1 = din("bc1", [64, 1], F32)
    wc2 = din("wc2", [64, 32], F32)
    bc2 = din("bc2", [32, 1], F32)

    outT = nc.dram_tensor("outT", [32, BS], F32, kind="ExternalOutput").ap()

    steps = [t for t in range(T) if n_sched[t] > 0]
    ns = len(steps)

    from contextlib import ExitStack

    with tile.TileContext(nc) as tc, ExitStack() as ctx:
        for _rep in range(reps):
            with ExitStack() as rctx:
                persist = rctx.enter_context(tc.tile_pool(name="persist", bufs=1))

                fw1T = persist.tile([H, TB], BF16, tag="fw1T")
                bw1T = persist.tile([H, TB], BF16, tag="bw1T")
                s_wrepf = persist.tile([H, 128], BF16, tag="wrepf")
                s_wrepb = persist.tile([H, 128], BF16, tag="wrepb")
                s_batt = persist.tile([128, 1], F32, tag="batt")
                nc.sync.dma_start(out=s_wrepf[:], in_=wrepf[:])
                nc.sync.dma_start(out=s_wrepb[:], in_=wrepb[:])
                nc.sync.dma_start(out=s_batt[:], in_=battc[:])

                s_cls = {}
                for nm, ap_, shp in (
                    ("xsT", xsT, [FS, BS]), ("ws0", ws0, [FS, 16]),
                    ("ws1", ws1, [16, 16]), ("bs0", bs0, [16, 1]),
                    ("bs1", bs1, [16, 1]), ("wc1s", wc1s, [16, 64]),
                    ("wc1f", wc1f, [H, 64]), ("wc1b", wc1b, [H, 64]),
                    ("bc1", bc1, [64, 1]), ("wc2", wc2, [64, 32]),
                    ("bc2", bc2, [32, 1]),
                ):
                    s_cls[nm] = persist.tile(shp, F32, tag="cls_" + nm, name="cls_" + nm)
                    nc.sync.dma_start(out=s_cls[nm][:], in_=ap_[:])

                sT = persist.tile([16, BS], F32, tag="sT")
                att_f = persist.tile([H, BS], F32, tag="att_f")
                att_b = persist.tile([H, BS], F32, tag="att_b")

                # ---- static branch (tiny) ----
                with tc.tile_pool(name="ps_static", bufs=1, space="PSUM") as pss:
                    ps1 = pss.tile([16, BS], F32, tag="pst1")
                    nc.tensor.matmul(ps1[:], s_cls["ws0"][:], s_cls["xsT"][:],
                                     start=True, stop=True)
                    s0 = persist.tile([16, BS], F32, tag="s0tmp")
                    nc.scalar.activation(s0[:], ps1[:], AF.Relu, bias=s_cls["bs0"][:])
                    ps2 = pss.tile([16, BS], F32, tag="pst2")
                    nc.tensor.matmul(ps2[:], s_cls["ws1"][:], s0[:], start=True, stop=True)
                    nc.scalar.activation(sT[:], ps2[:], AF.Relu, bias=s_cls["bs1"][:])

                # inactive slab cols must read 0 (attention pad semantics and
                # bw zero-prefix); zeros come via DMA, off the compute engines
                nc.sync.dma_start(out=fw1T[:], in_=zerosT[:])
                nc.sync.dma_start(out=bw1T[:], in_=zerosT[:])

                # ================= stages 0 and 1 =================
                # attention outputs (written whole by the post-pass reduces)
                den_acc = persist.tile([128, BS], F32, tag="den_acc")
                attf_acc = persist.tile([128, BS], F32, tag="attf_acc")
                attb_acc = persist.tile([128, BS], F32, tag="attb_acc")

                with ExitStack() as sctx:
                    p01 = sctx.enter_context(tc.tile_pool(name="stageA", bufs=1))
                    gates = sctx.enter_context(tc.tile_pool(name="gates", bufs=3))
                    psum = sctx.enter_context(
                        tc.tile_pool(name="psum01", bufs=4, space="PSUM"))

                    fw0T = p01.tile([H, CW], BF16, tag="fw0T")
                    bw0T = p01.tile([H, CWB], BF16, tag="bw0T")
                    s_ind2 = p01.tile([2, CW], BF16, tag="ind2")
                    hzero = p01.tile([H, BS], BF16, tag="hzero")
                    nc.gpsimd.memset(hzero[:], 0.0)
                    nc.sync.dma_start(out=bw0T[:], in_=zerosT[:, 0:CWB])
                    nc.sync.dma_start(out=s_ind2[:], in_=ind2[:])

                    s_w = {}
                    for nm, ap_, shp in (
                        ("wx0f", wx0f, [F + 2, 512]), ("wh0f", wh0f, [H, 512]),
                        ("wx0b", wx0b, [F + 2, 512]), ("wh0b", wh0b, [H, 512]),
                        ("wx1ft", wx1ft, [H, 512]), ("wx1fb", wx1fb, [H, 512]),
                        ("wh1f", wh1f, [H, 512]), ("wx1bt", wx1bt, [H, 512]),
                        ("wx1bb", wx1bb, [H, 512]), ("wh1b", wh1b, [H, 512]),
                        ("bias1f", bias1f, [2, 512]), ("bias1b", bias1b, [2, 512]),
                    ):
                        s_w[nm] = p01.tile(shp, BF16, tag="w_" + nm, name="w_" + nm)
                        nc.sync.dma_start(out=s_w[nm][:], in_=ap_[:])

                    # c-state: one [H, 2*BS] tile per stage per phase; fw in cols
                    # [0:BS], bw in cols [BS:2BS], so one tanh covers both directions.
                    cpair = {}
                    for st in ("0", "1"):
                        cpair[st] = [p01.tile([H, 2 * BS], F32, tag=f"cp_{st}_{k}",
                                              name=f"cp_{st}_{k}") for k in range(2)]
                        for k in range(2):
                            nc.gpsimd.memset(cpair[st][k][:], 0.0)
                    cstate = {
                        "f0": [cpair["0"][k][:, 0:BS] for k in range(2)],
                        "b0": [cpair["0"][k][:, BS:2 * BS] for k in range(2)],
                        "f1": [cpair["1"][k][:, 0:BS] for k in range(2)],
                        "b1": [cpair["1"][k][:, BS:2 * BS] for k in range(2)],
                    }

                    def lstm_step(name, idx, n, xmm, wh, out_ap, h_prev_ap, bias_mms):
                        """Matmuls + sigmoid + c update for one direction; returns the
                        sigmoid tile for the shared pair tail."""
                        cs = cstate[name]
                        c_prev, c_cur = cs[idx % 2], cs[(idx + 1) % 2]
                        ps = psum.tile([128, 512], F32, tag="ps")
                        # x contributions and bias rank-1s first (no dependency on the
                        # previous step's h), recurrent Wh matmuls last: PE drains the
                        # x-part while waiting for h, leaving only 4 matmuls on the
                        # recurrence critical path.
                        mms = []
                        for ch in range(4):
                            sl = slice(ch * 128, ch * 128 + n)
                            for w_, rhs in xmm:
                                mms.append((sl, w_[:, ch * 128:(ch + 1) * 128], rhs))
                        for bt, ch, ones_rhs in bias_mms:
                            mms.append((slice(ch * 128, ch * 128 + n),
                                        bt[:, ch * 128:(ch + 1) * 128], ones_rhs))
                        for ch in range(4):
                            sl = slice(ch * 128, ch * 128 + n)
                            mms.append((sl, wh[:, ch * 128:(ch + 1) * 128], h_prev_ap))
                        for i, (sl, lhsT, rhs) in enumerate(mms):
                            nc.tensor.matmul(ps[:, sl], lhsT, rhs,
                                             start=(i == 0), stop=(i == len(mms) - 1))
                        sig = gates.tile([128, 512], BF16, tag="sig")
                        ap_in = ps[:].rearrange("p (c n) -> p c n", c=4)[:, :, 0:n]
                        ap_out = sig[:].rearrange("p (c n) -> p c n", c=4)[:, :, 0:n]
                        nc.scalar.activation(ap_out, ap_in, AF.Sigmoid)
                        # c' = c*sig(f) + sig(i)*tanh(j), with tanh(j) = 2*sig(2j)-1
                        # (2x folded into weights):
                        #   t1 = sig(f) * c
                        #   u  = (sig(2j) - 0.5) * sig(i)      [= sig(i)*tanh(j)/2]
                        #   c' = 2*u + t1
                        t1 = gates.tile([128, BS], F32, tag="t1")
                        nc.gpsimd.tensor_tensor(t1[:, 0:n], sig[:, 128:128 + n],
                                                c_prev[:, 0:n], ALU.mult)
                        t2 = gates.tile([128, BS], BF16, tag="t2")
                        nc.vector.scalar_tensor_tensor(t2[:, 0:n], sig[:, 384:384 + n],
                                                       0.5, sig[:, 0:n],
                                                       ALU.subtract, ALU.mult)
                        nc.vector.scalar_tensor_tensor(c_cur[:, 0:n], t2[:, 0:n],
                                                       2.0, t1[:, 0:n],
                                                       ALU.mult, ALU.add)
                        tcn = gates.tile([128, BS], BF16, tag="tcn")
                        nc.scalar.activation(tcn[:, 0:n], c_cur[:, 0:n], AF.Tanh)
                        nc.vector.tensor_tensor(out_ap, tcn[:, 0:n],
                                                sig[:, 256:256 + n], ALU.mult)

                    # ---------- stage 0 (fw0 and bw0 interleaved) ----------
                    with tc.tile_pool(name="poolxd", bufs=1) as poolxd:
                        s_xdT = poolxd.tile([F + 2, CW], BF16, tag="xdT")
                        # xdT loaded in alternating head/tail chunks: fw needs the
                        # head first, bw needs the tail first.
                        NCH = 16
                        bounds = [round(i * CW / NCH) for i in range(NCH + 1)]
                        lo, hi = 0, NCH - 1
                        order = []
                        while lo <= hi:
                            order.append(lo)
                            if hi != lo:
                                order.append(hi)
                            lo += 1
                            hi -= 1
                        for ci in order:
                            a, b_ = bounds[ci], bounds[ci + 1]
                            if b_ > a:
                                nc.sync.dma_start(out=s_xdT[:, a:b_], in_=xdT[:, a:b_])

                        for k in range(ns):
                            t = steps[k]
                            n = n_sched[t]
                            hpf = (hzero[:, 0:n] if k == 0 else
                                   fw0T[:, off_fw[steps[k - 1]]:off_fw[steps[k - 1]] + n])
                            lstm_step(
                                "f0", k, n,
                                [(s_w["wx0f"], s_xdT[:, off_fw[t]:off_fw[t] + n])],
                                s_w["wh0f"], fw0T[:, off_fw[t]:off_fw[t] + n], hpf, [])
                            tb = steps[ns - 1 - k]
                            nb = n_sched[tb]
                            hpb = (hzero[:, 0:nb] if k == 0 else
                                   bw0T[:, off_bw[steps[ns - k]]:off_bw[steps[ns - k]] + nb])
                            lstm_step(
                                "b0", k, nb,
                                [(s_w["wx0b"], s_xdT[:, off_fw[tb]:off_fw[tb] + nb])],
                                s_w["wh0b"], bw0T[:, off_bw[tb]:off_bw[tb] + nb], hpb, [])

                    # ---------- stage 1 (fw1/bw1), pure recurrence ----------
                    for k in range(ns):
                        t = steps[k]
                        n = n_sched[t]
                        ones_f = s_ind2[0:2, off_fw[t]:off_fw[t] + n]
                        hpf = (hzero[:, 0:n] if k == 0 else
                               fw1T[:, steps[k - 1] * BS:steps[k - 1] * BS + n])
                        lstm_step(
                            "f1", k, n,
                            [(s_w["wx1ft"], fw0T[:, off_fw[t]:off_fw[t] + n]),
                             (s_w["wx1fb"], bw0T[:, off_bw[t]:off_bw[t] + n])],
                            s_w["wh1f"], fw1T[:, t * BS:t * BS + n], hpf,
                            [(s_w["bias1f"], ch, ones_f) for ch in bias1_chunks])
                        tb = steps[ns - 1 - k]
                        nb = n_sched[tb]
                        ones_b = s_ind2[0:2, off_fw[tb]:off_fw[tb] + nb]
                        hpb = (hzero[:, 0:nb] if k == 0 else
                               bw1T[:, steps[ns - k] * BS:steps[ns - k] * BS + nb])
                        lstm_step(
                            "b1", k, nb,
                            [(s_w["wx1bt"], fw0T[:, off_fw[tb]:off_fw[tb] + nb]),
                             (s_w["wx1bb"], bw0T[:, off_bw[tb]:off_bw[tb] + nb])],
                            s_w["wh1b"], bw1T[:, tb * BS:tb * BS + nb], hpb,
                            [(s_w["bias1b"], ch, ones_b) for ch in bias1_chunks])

                # ================= attention post-pass =================
                # All of it in a few big instructions after the recurrence:
                # logits matmuls -> tanh -> one giant exp -> in-place weighted
                # products -> log2-style fp16 pair-add rounds over t -> one
                # fp32 strided reduce per output. Inactive cols are zero in
                # the slabs, so th=0, es=1 there -- exactly the reference's
                # softmax-over-full-T semantics.
                with ExitStack() as actx:
                    pat = actx.enter_context(tc.tile_pool(name="attnp", bufs=1))
                    psa = actx.enter_context(
                        tc.tile_pool(name="psum_attn", bufs=2, space="PSUM"))
                    psr = actx.enter_context(
                        tc.tile_pool(name="psum_red", bufs=1, space="PSUM"))
                    es = pat.tile([128, TB], BF16, tag="es")
                    s_id = pat.tile([128, 128], BF16, tag="ident")
                    nc.sync.dma_start(out=s_id[:], in_=ident[:])
                    GC = 1024
                    for c0 in range(0, TB, GC):
                        gw = min(GC, TB - c0)
                        psl = psa.tile([128, GC], F32, tag="psl")
                        b0 = 0
                        while b0 < gw:
                            bw_ = min(512, gw - b0)
                            nc.tensor.matmul(psl[:, b0:b0 + bw_], s_wrepf[:],
                                             fw1T[:, c0 + b0:c0 + b0 + bw_],
                                             start=True, stop=False)
                            nc.tensor.matmul(psl[:, b0:b0 + bw_], s_wrepb[:],
                                             bw1T[:, c0 + b0:c0 + b0 + bw_],
                                             start=False, stop=True)
                            b0 += bw_
                        nc.scalar.activation(es[:, c0:c0 + gw], psl[:, 0:gw],
                                             AF.Tanh, bias=s_batt[:])
                    # exp + weighting in column chunks (ACT/DVE overlap), then
                    # the t-reduction as identity-matmul PSUM accumulation on
                    # the otherwise-idle PE (stationary loaded once per pass).
                    ps_red = {}
                    for nm in ("f", "b", "d"):
                        ps_red[nm] = psr.tile([128, BS], F32, tag="red_" + nm,
                                              name="red_" + nm)
                    NEXP = 4
                    TCH = (T + NEXP - 1) // NEXP
                    first = {"f": True, "b": True, "d": True}
                    for j in range(NEXP):
                        t0, t1_ = j * TCH, min((j + 1) * TCH, T)
                        c0, c1 = t0 * BS, t1_ * BS
                        nc.scalar.activation(es[:, c0:c1], es[:, c0:c1], AF.Exp)
                        nc.vector.tensor_tensor(fw1T[:, c0:c1], fw1T[:, c0:c1],
                                                es[:, c0:c1], ALU.mult)
                        nc.vector.tensor_tensor(bw1T[:, c0:c1], bw1T[:, c0:c1],
                                                es[:, c0:c1], ALU.mult)
                        for t in range(t0, t1_):
                            for nm, src in (("f", fw1T), ("b", bw1T), ("d", es)):
                                nc.tensor.matmul(
                                    ps_red[nm][:], s_id[:],
                                    src[:, t * BS:(t + 1) * BS],
                                    start=first[nm],
                                    stop=(t == T - 1))
                                first[nm] = False
                    nc.vector.tensor_copy(attf_acc[:], ps_red["f"][:])
                    nc.vector.tensor_copy(attb_acc[:], ps_red["b"][:])
                    nc.vector.tensor_copy(den_acc[:], ps_red["d"][:])

                # ================= normalize + classifier =================
                with ExitStack() as cctx:
                    pcl = cctx.enter_context(tc.tile_pool(name="cls", bufs=1))
                    psc = cctx.enter_context(
                        tc.tile_pool(name="psum_cls", bufs=1, space="PSUM"))
                    rd = pcl.tile([128, BS], F32, tag="rd")
                    nc.vector.reciprocal(rd[:], den_acc[:])
                    nc.vector.tensor_tensor(att_f[:], attf_acc[:], rd[:], ALU.mult)
                    nc.vector.tensor_tensor(att_b[:], attb_acc[:], rd[:], ALU.mult)

                    ph = psc.tile([64, BS], F32, tag="ph")
                    nc.tensor.matmul(ph[:], s_cls["wc1s"][:], sT[:], start=True,
                                     stop=False)
                    nc.tensor.matmul(ph[:], s_cls["wc1f"][:], att_f[:], start=False,
                                     stop=False)
                    nc.tensor.matmul(ph[:], s_cls["wc1b"][:], att_b[:], start=False,
                                     stop=True)
                    h1 = pcl.tile([64, BS], F32, tag="h1")
                    nc.scalar.activation(h1[:], ph[:], AF.Relu, bias=s_cls["bc1"][:])
                    po = psc.tile([32, BS], F32, tag="po")
                    nc.tensor.matmul(po[:], s_cls["wc2"][:], h1[:], start=True,
                                     stop=True)
                    oT = pcl.tile([32, BS], F32, tag="oT")
                    nc.scalar.activation(oT[:], po[:], AF.Relu, bias=s_cls["bc2"][:])
                    nc.sync.dma_start(out=outT[:], in_=oT[:])

    return nc


KILL = -30.0  # sigma(-30) ~ 1e-13: forces h,c to exact 0 (fp16) for ended rows


def prepare(inputs):
    """Compute the per-call schedule and the 8 per-core input maps."""
    L = np.asarray(inputs["seq_len"]).astype(np.int64)
    order = np.argsort(-L, kind="stable")
    perms = [order[c::NCORES] for c in range(NCORES)]

    n_true = np.zeros((NCORES, T), dtype=np.int64)
    for c in range(NCORES):
        Lc = L[perms[c]]
        n_true[c] = (Lc[None, :].T > np.arange(T)[None, :]).sum(axis=0)
    n_sched = n_true.max(axis=0)
    n_sched = np.minimum(((n_sched + 3) // 4) * 4, BS).astype(np.int64)

    off_fw = np.zeros(T, dtype=np.int64)
    acc = 0
    for t in range(T):
        off_fw[t] = acc
        acc += int(n_sched[t])
    CW = int(max(acc, 4))
    off_bw = np.zeros(T, dtype=np.int64)
    acc = 0
    for t in range(T):
        off_bw[t] = acc
        acc += int(max(n_sched[t], n_sched[t - 1] if t > 0 else 0))
    CWB = int(max(acc, 4))

    # layer-1 rank-1 rows: row0 = bias (+forget 1), row1 = kill row
    kill_row = np.zeros(512)
    kill_row[0:384] = KILL  # i, f, o chunks; j untouched
    b1f_row = np.asarray(inputs["bb_f1"], dtype=np.float64)[_GATE_PERM].copy()
    b1b_row = np.asarray(inputs["bb_b1"], dtype=np.float64)[_GATE_PERM].copy()
    b1f_row[H:2 * H] += 1.0
    b1b_row[H:2 * H] += 1.0
    bias_chunks = [ch for ch in range(4)
                   if np.any(b1f_row[ch * H:(ch + 1) * H] != 0)
                   or np.any(b1b_row[ch * H:(ch + 1) * H] != 0)]
    l1_chunks = sorted(set(bias_chunks) | {0, 1, 2})

    def jscale(w):
        w = np.array(w, dtype=np.float64)
        w[..., 3 * H:4 * H] *= 2.0  # fold tanh(j)=2*sigma(2j)-1
        return w

    def stage_wx0(wx, bb):
        w = np.asarray(wx, dtype=np.float64)[:, _GATE_PERM]
        bias = np.asarray(bb, dtype=np.float64)[_GATE_PERM].copy()
        bias[H:2 * H] += 1.0
        return _bf16(jscale(np.concatenate([bias[None, :], kill_row[None, :], w],
                                           axis=0)))

    base_map = dict(
        zerosT=np.zeros((128, T * BS), dtype=np.float16),
        ident=np.eye(128, dtype=np.float16),
        wx0f=stage_wx0(inputs["Wx_f0"], inputs["bb_f0"]),
        wx0b=stage_wx0(inputs["Wx_b0"], inputs["bb_b0"]),
        wh0f=_bf16(jscale(np.asarray(inputs["Wh_f0"])[:, _GATE_PERM])),
        wh0b=_bf16(jscale(np.asarray(inputs["Wh_b0"])[:, _GATE_PERM])),
        wx1ft=_bf16(jscale(np.asarray(inputs["Wx_f1"])[0:H, _GATE_PERM])),
        wx1fb=_bf16(jscale(np.asarray(inputs["Wx_f1"])[H:2 * H, _GATE_PERM])),
        wh1f=_bf16(jscale(np.asarray(inputs["Wh_f1"])[:, _GATE_PERM])),
        wx1bt=_bf16(jscale(np.asarray(inputs["Wx_b1"])[0:H, _GATE_PERM])),
        wx1bb=_bf16(jscale(np.asarray(inputs["Wx_b1"])[H:2 * H, _GATE_PERM])),
        wh1b=_bf16(jscale(np.asarray(inputs["Wh_b1"])[:, _GATE_PERM])),
        bias1f=_bf16(jscale(np.stack([b1f_row, kill_row]))),
        bias1b=_bf16(jscale(np.stack([b1b_row, kill_row]))),
        wrepf=_bf16(np.repeat(np.asarray(inputs["w_att"])[0:H, 0:1], 128, axis=1)),
        wrepb=_bf16(np.repeat(np.asarray(inputs["w_att"])[H:2 * H, 0:1], 128, axis=1)),
        battc=_f32(np.full((128, 1),
                           float(np.asarray(inputs["b_att"]).reshape(-1)[0]))),
        ws0=_f32(inputs["w_s0"]), ws1=_f32(inputs["w_s1"]),
        bs0=_f32(np.asarray(inputs["b_s0"]).reshape(-1, 1)),
        bs1=_f32(np.asarray(inputs["b_s1"]).reshape(-1, 1)),
        wc1s=_f32(np.asarray(inputs["w_c1"])[0:16]),
        wc1f=_f32(np.asarray(inputs["w_c1"])[16:16 + H]),
        wc1b=_f32(np.asarray(inputs["w_c1"])[16 + H:16 + 2 * H]),
        bc1=_f32(np.asarray(inputs["b_c1"]).reshape(-1, 1)),
        wc2=_f32(inputs["w_c2"]),
        bc2=_f32(np.asarray(inputs["b_c2"]).reshape(-1, 1)),
    )

    in_maps = []
    for c in range(NCORES):
        p = perms[c]
        Lc = L[p]
        xc = np.asarray(inputs["x_dynamic"])[p].astype(np.float32)
        tmask = (np.arange(T)[None, :] < Lc[:, None])
        xc = np.where(tmask[:, :, None], xc, 0.0)
        xcT = xc.transpose(2, 1, 0)
        xdT_h = np.zeros((F + 2, CW), dtype=np.float32)
        for t in range(T):
            n = int(n_sched[t])
            if n == 0:
                continue
            o = int(off_fw[t])
            nt = min(int(n_true[c, t]), n)
            xdT_h[2:F + 2, o:o + n] = xcT[:, t, 0:n]
            xdT_h[0, o:o + nt] = 1.0
            xdT_h[1, o + nt:o + n] = 1.0
        m = dict(base_map)
        m["xdT"] = _bf16(xdT_h)
        m["ind2"] = _bf16(xdT_h[0:2])
        m["xsT"] = _f32(np.asarray(inputs["x_static"])[p].T)
        in_maps.append(m)

    sched = dict(n_sched=n_sched, off_fw=off_fw, off_bw=off_bw, CW=CW,
                 CWB=CWB, l1_chunks=l1_chunks, perms=perms, n_true=n_true)
    return sched, in_maps


def kernel(x_static, x_dynamic, seq_len, w_s0, b_s0, w_s1, b_s1,
           Wx_f0, Wh_f0, bb_f0, Wx_b0, Wh_b0, bb_b0,
           Wx_f1, Wh_f1, bb_f1, Wx_b1, Wh_b1, bb_b1,
           w_att, b_att, w_c1, b_c1, w_c2, b_c2):
    inputs = dict(
        x_static=x_static, x_dynamic=x_dynamic, seq_len=seq_len,
        w_s0=w_s0, b_s0=b_s0, w_s1=w_s1, b_s1=b_s1,
        Wx_f0=Wx_f0, Wh_f0=Wh_f0, bb_f0=bb_f0,
        Wx_b0=Wx_b0, Wh_b0=Wh_b0, bb_b0=bb_b0,
        Wx_f1=Wx_f1, Wh_f1=Wh_f1, bb_f1=bb_f1,
        Wx_b1=Wx_b1, Wh_b1=Wh_b1, bb_b1=bb_b1,
        w_att=w_att, b_att=b_att, w_c1=w_c1, b_c1=b_c1,
        w_c2=w_c2, b_c2=b_c2,
    )
    sched, in_maps = prepare(inputs)
    nc = build_program(sched["n_sched"], sched["off_fw"], sched["off_bw"],
                       sched["CW"], sched["CWB"], sched["l1_chunks"])
    split_multi_waits(nc, max_waits=1)

    trace = os.environ.get("TRN_KERNEL_TRACE", "0") == "1"
    try:
        res = run_bass_kernel_spmd(nc, in_maps, list(range(NCORES)), trace=trace)
    except ModuleNotFoundError:
        # NTFF profiling hook unavailable in this container
        res = run_bass_kernel_spmd(nc, in_maps, list(range(NCORES)))
    if trace:
        kernel.last_results = res
        print(f"[kernel] exec_time_ns={res.exec_time_ns} "
              f"mean={res.mean_exec_time_ns}")

    out = np.zeros((B, 32), dtype=np.float32)
    for c in range(NCORES):
        out[sched["perms"][c]] = res.results[c]["outT"].T
    return out

